# revision 5
# baseline (speedup 1.0000x reference)
"""Trainium2 Bass kernel for y = -x + (A @ x^2) / (x^2 + 1).

A [16384, 16384] f32 is sharded row-wise across 8 NeuronCores (2048
output rows per core). Each core receives the *transposed* slice
AT_c = A[rows].T ([16384, 2048], contiguous) so the contraction index j
lands on SBUF partitions. The core streams AT_c once from HBM in 1 MiB
tiles (memory-bound roofline), multiplies against a resident x^2 table
with float32r matmuls (N=512, full-rate on the PE), and applies the
elementwise epilogue on the transposed [16, 2048] output, which the host
transposes back and concatenates.
"""

import numpy as np

import concourse.bacc as bacc
import concourse.tile as tile
from concourse import mybir
from concourse.bass_utils import run_bass_kernel_spmd

N_NODES = 16384
DIM = 16
N_CORES = 8
ROWS = N_NODES // N_CORES      # 2048 output rows per core
P = 128                        # SBUF partitions / matmul contraction tile
JB = N_NODES // P              # 128 contraction blocks
NCHUNK = 512                   # matmul moving free dim (one PSUM bank)
ICN = ROWS // NCHUNK           # 4 output column chunks per core

f32 = mybir.dt.float32
f32r = mybir.dt.float32r


def build_program(reps: int = 1, a_bufs: int = 8):
    nc = bacc.Bacc(
        "TRN2", target_bir_lowering=False, debug=False, num_devices=N_CORES
    )
    at_d = nc.dram_tensor("at", [N_NODES, ROWS], f32r, kind="ExternalInput")
    xq_d = nc.dram_tensor("xq", [P, JB * DIM], f32, kind="ExternalInput")
    xt_d = nc.dram_tensor("xt", [DIM, ROWS], f32, kind="ExternalInput")
    yt_d = nc.dram_tensor("yt", [DIM, ROWS], f32, kind="ExternalOutput")

    with tile.TileContext(nc) as tc:
        with (
            tc.tile_pool(name="const", bufs=1) as const_pool,
            tc.tile_pool(name="a", bufs=a_bufs) as a_pool,
            tc.tile_pool(name="ps", bufs=1, space="PSUM") as ps_pool,
            tc.tile_pool(name="y", bufs=2) as y_pool,
        ):
            # Resident x^2 table: xh[p, jb*16 + d] = x[jb*128 + p, d]^2,
            # so xh[:, jb*16:(jb+1)*16] is the [K=128, M=16] lhsT for
            # contraction block jb.
            xload = const_pool.tile([P, JB * DIM], f32, tag="xload")
            nc.sync.dma_start(xload[:], xq_d.ap())
            xh = const_pool.tile([P, JB * DIM], f32r, tag="xh")
            nc.vector.tensor_mul(xh[:], xload[:], xload[:])

            # Epilogue constants on the local row slice (transposed):
            # xt[d, f] = x[c*2048 + f, d], rcp = 1 / (xt^2 + 1).
            xt = const_pool.tile([DIM, ROWS], f32, tag="xt")
            nc.sync.dma_start(xt[:], xt_d.ap())
            rcp = const_pool.tile([DIM, ROWS], f32, tag="rcp")
            nc.vector.tensor_mul(rcp[:], xt[:], xt[:])
            nc.scalar.add(rcp[:], rcp[:], 1.0)
            nc.vector.reciprocal(rcp[:], rcp[:])

            at_blocks = at_d.ap().rearrange("(jb p) i -> jb p i", p=P)
            for _ in range(reps):
                ps = [
                    ps_pool.tile([DIM, NCHUNK], f32,
                                 name=f"ps{ic}", tag=f"ps{ic}")
                    for ic in range(ICN)
                ]
                for jb in range(JB):
                    a_t = a_pool.tile([P, ROWS], f32r, tag="a")
                    nc.sync.dma_start(a_t[:], at_blocks[jb, :, :])
                    lhsT = xh[:, jb * DIM:(jb + 1) * DIM]
                    for ic in range(ICN):
                        nc.tensor.matmul(
                            ps[ic][:],
                            lhsT,
                            a_t[:, ic * NCHUNK:(ic + 1) * NCHUNK],
                            start=(jb == 0),
                            stop=(jb == JB - 1),
                        )
                for ic in range(ICN):
                    sl = slice(ic * NCHUNK, (ic + 1) * NCHUNK)
                    y_t = y_pool.tile([DIM, NCHUNK], f32, tag="y")
                    nc.vector.tensor_mul(y_t[:], ps[ic][:], rcp[:, sl])
                    nc.vector.tensor_sub(y_t[:], y_t[:], xt[:, sl])
                    nc.sync.dma_start(yt_d.ap()[:, sl], y_t[:])
    nc.compile()
    return nc


def shard_inputs(A: np.ndarray, x: np.ndarray) -> list[dict]:
    A = np.ascontiguousarray(A, dtype=np.float32)
    x = np.ascontiguousarray(x, dtype=np.float32)
    xq = np.ascontiguousarray(
        x.reshape(JB, P, DIM).transpose(1, 0, 2)
    ).reshape(P, JB * DIM)
    in_maps = []
    for c in range(N_CORES):
        rows = slice(c * ROWS, (c + 1) * ROWS)
        in_maps.append({
            "at": np.ascontiguousarray(A[rows, :].T),
            "xq": xq,
            "xt": np.ascontiguousarray(x[rows, :].T),
        })
    return in_maps


def gather_output(results: list[dict]) -> np.ndarray:
    return np.concatenate(
        [np.asarray(results[c]["yt"]).T for c in range(N_CORES)], axis=0
    ).astype(np.float32)


def kernel(A, x, t=None, **_unused) -> np.ndarray:
    nc = build_program(reps=1)
    in_maps = shard_inputs(np.asarray(A), np.asarray(x))
    res = run_bass_kernel_spmd(nc, in_maps, core_ids=list(range(N_CORES)))
    return gather_output(res.results)


# revision 7
# speedup vs baseline: 1.0450x; 1.0450x over previous
"""Trainium2 Bass kernel for y = -x + (A @ x^2) / (x^2 + 1).

A [16384, 16384] f32 is sharded row-wise across 8 NeuronCores (2048
output rows per core). Each core receives the *transposed* slice
AT_c = A[rows].T ([16384, 2048], contiguous) so the contraction index j
lands on SBUF partitions. The core streams AT_c once from HBM in 1 MiB
tiles (memory-bound roofline), multiplies against a resident x^2 table
with float32r matmuls (N=512, full-rate on the PE), and applies the
elementwise epilogue on the transposed [16, 2048] output, which the host
transposes back and concatenates.
"""

import numpy as np

import concourse.bacc as bacc
import concourse.tile as tile
from concourse import mybir
from concourse.bass_utils import run_bass_kernel_spmd

N_NODES = 16384
DIM = 16
N_CORES = 8
ROWS = N_NODES // N_CORES      # 2048 output rows per core
P = 128                        # SBUF partitions / matmul contraction tile
JB = N_NODES // P              # 128 contraction blocks
NCHUNK = 512                   # matmul moving free dim (one PSUM bank)
ICN = ROWS // NCHUNK           # 4 output column chunks per core

f32 = mybir.dt.float32
f32r = mybir.dt.float32r


def build_program(reps: int = 1, a_bufs: int = 8, jb_per_tile: int = 1):
    nc = bacc.Bacc(
        "TRN2", target_bir_lowering=False, debug=False, num_devices=N_CORES
    )
    at_d = nc.dram_tensor("at", [N_NODES, ROWS], f32r, kind="ExternalInput")
    xq_d = nc.dram_tensor("xq", [P, JB * DIM], f32, kind="ExternalInput")
    xt_d = nc.dram_tensor("xt", [DIM, ROWS], f32, kind="ExternalInput")
    yt_d = nc.dram_tensor("yt", [DIM, ROWS], f32, kind="ExternalOutput")

    with tile.TileContext(nc) as tc:
        with (
            tc.tile_pool(name="const", bufs=1) as const_pool,
            tc.tile_pool(name="a", bufs=a_bufs) as a_pool,
            tc.tile_pool(name="ps", bufs=1, space="PSUM") as ps_pool,
            tc.tile_pool(name="y", bufs=2) as y_pool,
        ):
            # Resident x^2 table: xh[p, jb*16 + d] = x[jb*128 + p, d]^2,
            # so xh[:, jb*16:(jb+1)*16] is the [K=128, M=16] lhsT for
            # contraction block jb.
            xload = const_pool.tile([P, JB * DIM], f32, tag="xload")
            nc.sync.dma_start(xload[:], xq_d.ap())
            xh = const_pool.tile([P, JB * DIM], f32r, tag="xh")
            nc.vector.tensor_mul(xh[:], xload[:], xload[:])

            # Epilogue constants on the local row slice (transposed):
            # xt[d, f] = x[c*2048 + f, d], rcp = 1 / (xt^2 + 1).
            xt = const_pool.tile([DIM, ROWS], f32, tag="xt")
            nc.sync.dma_start(xt[:], xt_d.ap())
            rcp = const_pool.tile([DIM, ROWS], f32, tag="rcp")
            nc.vector.tensor_mul(rcp[:], xt[:], xt[:])
            nc.scalar.add(rcp[:], rcp[:], 1.0)
            nc.vector.reciprocal(rcp[:], rcp[:])

            # [n_tiles, 128, jb_per_tile * ROWS]: each slice along dim 0 is
            # a contiguous (jb_per_tile MiB) run of jb_per_tile j-blocks.
            at_blocks = at_d.ap().rearrange(
                "(t g p) i -> t p (g i)", p=P, g=jb_per_tile
            )
            n_tiles = JB // jb_per_tile
            for _ in range(reps):
                ps = [
                    ps_pool.tile([DIM, NCHUNK], f32,
                                 name=f"ps{ic}", tag=f"ps{ic}")
                    for ic in range(ICN)
                ]
                for ti in range(n_tiles):
                    a_t = a_pool.tile([P, jb_per_tile * ROWS], f32r, tag="a")
                    nc.sync.dma_start(a_t[:], at_blocks[ti, :, :])
                    for g in range(jb_per_tile):
                        jb = ti * jb_per_tile + g
                        lhsT = xh[:, jb * DIM:(jb + 1) * DIM]
                        for ic in range(ICN):
                            nc.tensor.matmul(
                                ps[ic][:],
                                lhsT,
                                a_t[:, g * ROWS + ic * NCHUNK:
                                       g * ROWS + (ic + 1) * NCHUNK],
                                start=(jb == 0),
                                stop=(jb == JB - 1),
                            )
                for ic in range(ICN):
                    sl = slice(ic * NCHUNK, (ic + 1) * NCHUNK)
                    y_t = y_pool.tile([DIM, NCHUNK], f32, tag="y")
                    nc.vector.tensor_mul(y_t[:], ps[ic][:], rcp[:, sl])
                    nc.vector.tensor_sub(y_t[:], y_t[:], xt[:, sl])
                    nc.sync.dma_start(yt_d.ap()[:, sl], y_t[:])
    nc.compile()
    return nc


def shard_inputs(A: np.ndarray, x: np.ndarray) -> list[dict]:
    A = np.ascontiguousarray(A, dtype=np.float32)
    x = np.ascontiguousarray(x, dtype=np.float32)
    xq = np.ascontiguousarray(
        x.reshape(JB, P, DIM).transpose(1, 0, 2)
    ).reshape(P, JB * DIM)
    in_maps = []
    for c in range(N_CORES):
        rows = slice(c * ROWS, (c + 1) * ROWS)
        in_maps.append({
            "at": np.ascontiguousarray(A[rows, :].T),
            "xq": xq,
            "xt": np.ascontiguousarray(x[rows, :].T),
        })
    return in_maps


def gather_output(results: list[dict]) -> np.ndarray:
    return np.concatenate(
        [np.asarray(results[c]["yt"]).T for c in range(N_CORES)], axis=0
    ).astype(np.float32)


def kernel(A, x, t=None, **_unused) -> np.ndarray:
    nc = build_program(reps=1)
    in_maps = shard_inputs(np.asarray(A), np.asarray(x))
    res = run_bass_kernel_spmd(nc, in_maps, core_ids=list(range(N_CORES)))
    return gather_output(res.results)
